# revision 6
# baseline (speedup 1.0000x reference)
"""Trainium2 Bass kernel for chunked recurrent causal linear attention.

Problem: b=2, h=8, n=2048, d=128, e=64, chunk=128, two branches (plain +
rotary) sharing one denominator.

Math (per (b,h), per chunk c, token t in chunk, with running state
S[d,e], Z[d] per branch):
    AT[s,t]   = k_s . q_t                  (s,t in chunk; masked to s<=t)
    num[t,:]  = sum_s ATm[s,t] v_s + q_t @ S      (both branches summed)
    den[t]    = sum_s ATm[s,t]   + q_t . Z        (both branches summed)
    out[t,:]  = num[t,:] / den[t]
    S += k_chunk^T v_chunk ;  Z += sum_s k_s

Sharding: 16 (b,h) pairs over 8 cores, 2 pairs per core. Host ships
pre-transposed copies of q/k/q_rot/k_rot (so no on-device transposes are
needed) plus natural-layout k/k_rot (stationary operand of the state
update) and v with a ones-column appended (fuses the denominator into
the numerator matmuls).
"""

import contextlib
import sys

_nullctx = contextlib.nullcontext

if "/opt/trn_rl_repo" not in sys.path:
    sys.path.insert(0, "/opt/trn_rl_repo")

import numpy as np

import concourse.bass as bass
import concourse.tile as tile
from concourse import bacc, mybir
from concourse.bass_utils import run_bass_kernel_spmd

F32 = mybir.dt.float32
F32R = mybir.dt.float32r

N_CORES = 8
PAIRS_PER_CORE = 2
N = 2048           # sequence length per (b,h)
D = 128            # qk head dim
E = 64             # v head dim
E1 = E + 1         # v plus ones column
C = 128            # chunk size
NCHUNK = N // C    # 16
SLAB = 4           # chunks per DMA slab
SLAB_BUFS = 6      # slab pool buffers
NROWS = PAIRS_PER_CORE * N  # 4096

_cached = {}


def build_kernel(repeat=1, loop_k=None, dma_only=False, reuse_slab=False,
                 probe_no_at=False, probe_no_state=False, transpose_k=False,
                 pipe=1, host_norm=False, dma_split=False, taper=False,
                 big_bufs=False, load_reorder=False, bank_42=False,
                 stagger=False, probe_pe_only=False, mm_f32r=False,
                 f32r=False, fast_start=False, ilv=True, dt16=True):
    nc = bacc.Bacc("TRN2", target_bir_lowering=False, debug=False,
                   num_devices=N_CORES)

    # fp16 inputs: 2x less DMA traffic and 4x PE matmul throughput vs fp32
    # (fp32 matmuls lower to 2 half-speed passes). PSUM accumulation stays
    # fp32; measured end-to-end rel err 4.3e-4 vs the 2e-2 gate.
    MT = mybir.dt.float16 if dt16 else F32

    def mm(out_ap, lhsT_ap, rhs_ap, **kw):
        if mm_f32r:
            lhsT_ap = lhsT_ap.bitcast(F32R)
            rhs_ap = rhs_ap.bitcast(F32R)
        return nc.tensor.matmul(out_ap, lhsT_ap, rhs_ap, **kw)

    qT = nc.dram_tensor("qT", [D, NROWS], MT, kind="ExternalInput").ap()
    kT = nc.dram_tensor("kT", [D, NROWS], MT, kind="ExternalInput").ap()
    qrT = nc.dram_tensor("qrT", [D, NROWS], MT, kind="ExternalInput").ap()
    krT = nc.dram_tensor("krT", [D, NROWS], MT, kind="ExternalInput").ap()
    if not transpose_k:
        kn = nc.dram_tensor("kn", [NROWS, D], MT, kind="ExternalInput").ap()
        krn = nc.dram_tensor("krn", [NROWS, D], MT, kind="ExternalInput").ap()
    else:
        ident = nc.dram_tensor("ident", [C, C], F32, kind="ExternalInput").ap()
    v1 = nc.dram_tensor("v1", [NROWS, E1], MT, kind="ExternalInput").ap()
    mask = nc.dram_tensor("mask", [C, C], F32, kind="ExternalInput").ap()
    EO = E1 if host_norm else E
    OD = F32 if host_norm else MT  # fp16 out halves write traffic; host upcasts
    out = nc.dram_tensor("out", [NROWS, EO], OD, kind="ExternalOutput").ap()

    if taper:
        plans = [{0: 2, 2: 4, 6: 4, 10: 4, 14: 2}] * PAIRS_PER_CORE
    elif stagger:
        plans = [{c0: SLAB for c0 in range(0, NCHUNK, SLAB)},
                 {0: 2, 2: 4, 6: 4, 10: 4, 14: 2}]
    else:
        plans = [{c0: SLAB for c0 in range(0, NCHUNK, SLAB)}] * PAIRS_PER_CORE
    slab_of = []
    for pp in range(PAIRS_PER_CORE):
        m = {}
        for c0, ln in plans[pp].items():
            for c in range(c0, c0 + ln):
                m[c] = (c0, ln)
        slab_of.append(m)

    with tile.TileContext(nc) as tc:
        with (
            tc.tile_pool(name="const", bufs=1) as constp,
            tc.tile_pool(name="slabs", bufs=SLAB_BUFS) as slabs,
            tc.tile_pool(name="atm", bufs=(6 if big_bufs else (4 if pipe == 1 else 6))) as atmp,
            tc.tile_pool(name="ssb", bufs=(12 if big_bufs else 8)) as ssbp,
            tc.tile_pool(name="dinv", bufs=(12 if big_bufs else 8)) as dinvp,
            tc.tile_pool(name="pat", bufs=(2 if (transpose_k or bank_42) else 3),
                         space="PSUM") as patp,
            tc.tile_pool(name="pout", bufs=(2 if transpose_k else (4 if bank_42 else 3)),
                         space="PSUM") as poutp,
            tc.tile_pool(name="pst", bufs=2, space="PSUM") as pstp,
            tc.tile_pool(name="ktr", bufs=2, space="PSUM") as ktrp,
            tc.tile_pool(name="kns", bufs=4) as knsp,
        ):
            mask_t = constp.tile([C, C], F32, tag="mask")
            nc.sync.dma_start(mask_t[:], mask[:])
            if transpose_k:
                ident_t = constp.tile([C, C], F32, tag="ident")
                nc.sync.dma_start(ident_t[:], ident[:])

            for rep in range(repeat):
              with (tc.For_i(0, loop_k, 1, hint_engines=(
                        mybir.EngineType.PE, mybir.EngineType.DVE,
                        mybir.EngineType.Activation, mybir.EngineType.SP))
                    if (loop_k is not None and loop_k > 1)
                    else _nullctx()):
                  # per-pair state accumulator in one PSUM bank:
                  # cols 0:65 -> branch 0 [S|Z], cols 66:131 -> branch 1
                  pS = {}
                  for p in range(PAIRS_PER_CORE):
                      pS[p] = pstp.tile([D, 2 * E1 + 2], F32, tag="pS", name=f"pS_{rep}_{p}")

                  slab_t = [None] * PAIRS_PER_CORE   # per pair: dict of slab tiles
                  S_sbuf = {}                        # (p, br) -> sbuf state tile

                  # Software pipeline, one chunk deep: the "front" stage of
                  # chunk c emits loads, the state update (PE), and AT+mask
                  # (PE then DVE); the "back" stage consumes chunk c-1's
                  # masked AT for the numerator/denominator matmuls. This
                  # gives every cross-engine hop a full stage of slack, so
                  # the PE never head-of-line blocks on DVE/ACT latency.
                  fifo = []
                  for cc in range(NCHUNK + pipe):
                    pending = {}
                    back = {}
                    if cc >= pipe:
                        back = fifo.pop(0)
                    if cc < NCHUNK:
                        fifo.append(pending)
                    if cc < NCHUNK:
                      c = cc
                      for p in range(PAIRS_PER_CORE):
                          c0, slen = slab_of[p][c]
                          if (c == c0) and not (reuse_slab and c > 0):
                              base = p * N + c * C
                              cols = slice(base, base + slen * C)
                              dmae = nc.gpsimd if (dma_split and p == 1) else nc.sync
                              st = {"len": slen}
                              if not load_reorder:
                                  st["qT"] = slabs.tile([D, slen * C], MT, tag="qT", name=f"qTs_{rep}_{p}_{c}")
                                  st["kT"] = slabs.tile([D, slen * C], MT, tag="kT", name=f"kTs_{rep}_{p}_{c}")
                                  if fast_start and c == 0:
                                      # split the very first q/k loads so chunk
                                      # 0's AT matmul starts after 128KB, not
                                      # a full slab (range-level tile deps)
                                      dmae.dma_start(st["qT"][:, 0:C], qT[:, base:base + C])
                                      dmae.dma_start(st["kT"][:, 0:C], kT[:, base:base + C])
                                      dmae.dma_start(st["qT"][:, C:slen * C], qT[:, base + C:base + slen * C])
                                      dmae.dma_start(st["kT"][:, C:slen * C], kT[:, base + C:base + slen * C])
                                  else:
                                      dmae.dma_start(st["qT"][:], qT[:, cols])
                                      dmae.dma_start(st["kT"][:], kT[:, cols])
                                  st["qrT"] = slabs.tile([D, slen * C], MT, tag="qrT", name=f"qrTs_{rep}_{p}_{c}")
                                  dmae.dma_start(st["qrT"][:], qrT[:, cols])
                                  st["krT"] = slabs.tile([D, slen * C], MT, tag="krT", name=f"krTs_{rep}_{p}_{c}")
                                  dmae.dma_start(st["krT"][:], krT[:, cols])
                              # load the state-update inputs (kn/krn/v1)
                              # first: they feed the first PE ops of the chunk
                              if not transpose_k:
                                  st["kn"] = slabs.tile([C, slen, D], MT, tag="kn", name=f"kns_{rep}_{p}_{c}")
                                  dmae.dma_start(
                                      st["kn"][:],
                                      kn[cols, :].rearrange("(n p) d -> p n d", p=C))
                                  st["krn"] = slabs.tile([C, slen, D], MT, tag="krn", name=f"krns_{rep}_{p}_{c}")
                                  dmae.dma_start(
                                      st["krn"][:],
                                      krn[cols, :].rearrange("(n p) d -> p n d", p=C))
                              st["v1"] = slabs.tile([C, slen, E1], MT, tag="v1", name=f"v1s_{rep}_{p}_{c}")
                              dmae.dma_start(
                                  st["v1"][:],
                                  v1[cols, :].rearrange("(n p) e -> p n e", p=C))
                              if load_reorder:
                                  st["kT"] = slabs.tile([D, slen * C], MT, tag="kT", name=f"kTs_{rep}_{p}_{c}")
                                  dmae.dma_start(st["kT"][:], kT[:, cols])
                                  st["qT"] = slabs.tile([D, slen * C], MT, tag="qT", name=f"qTs_{rep}_{p}_{c}")
                                  dmae.dma_start(st["qT"][:], qT[:, cols])
                                  st["qrT"] = slabs.tile([D, slen * C], MT, tag="qrT", name=f"qrTs_{rep}_{p}_{c}")
                                  dmae.dma_start(st["qrT"][:], qrT[:, cols])
                                  st["krT"] = slabs.tile([D, slen * C], MT, tag="krT", name=f"krTs_{rep}_{p}_{c}")
                                  dmae.dma_start(st["krT"][:], krT[:, cols])
                              st["outs"] = slabs.tile([C, slen, EO], OD, tag="outs", name=f"outs_{rep}_{p}_{c}")
                              slab_t[p] = st

                          st = slab_t[p]
                          j = c - c0
                          qcT = st["qT"][:, j * C:(j + 1) * C]
                          kcT = st["kT"][:, j * C:(j + 1) * C]
                          qrcT = st["qrT"][:, j * C:(j + 1) * C]
                          krcT = st["krT"][:, j * C:(j + 1) * C]
                          vc = st["v1"][:, j, :]
                          knc = krnc = None
                          if not transpose_k:
                              knc = st["kn"][:, j, :]
                              krnc = st["krn"][:, j, :]

                          if dma_only:
                              continue

                          if probe_pe_only:
                              # pure matmul throughput probe: same 7 MMs as the
                              # real kernel, but no cross-engine deps at all
                              pat0 = patp.tile([C, C], F32, tag="pat")
                              mm(pat0[:], kcT, qcT, start=True, stop=False)
                              mm(pat0[:], krcT, qrcT, start=False, stop=True)
                              po = poutp.tile([C, E1], F32, tag="po")
                              mm(po[:], mask_t[:], vc, start=True, stop=False)
                              mm(po[:], qcT, mask_t[:, 0:E1], start=False, stop=False, skip_group_check=True)
                              mm(po[:], qrcT, mask_t[:, 0:E1], start=False, stop=True, skip_group_check=True)
                              mm(pS[p][:, 0:E1], knc, vc, start=(c == 0), stop=False, skip_group_check=True)
                              mm(pS[p][:, E1 + 1:2 * E1 + 1], krnc, vc, start=False, stop=(c == NCHUNK - 1), skip_group_check=True)
                              continue

                          prev_S = S_sbuf.get(p)

                          if ilv:
                              # MMs emitted pair-interleaved after this loop
                              pending[p] = dict(qcT=qcT, qrcT=qrcT, kcT=kcT,
                                                krcT=krcT, knc=knc, krnc=krnc,
                                                vc=vc, st=st, j=j, c=c,
                                                prev_S=prev_S, kns0=None,
                                                kns1=None, c0=c0,
                                                slen=st.get("len", SLAB))
                              continue

                          # State update: both branches share one PSUM bank
                          # (start=True on c0/br0 clears it; br1 overwrites its
                          # unwritten columns). Without transpose_k the natural-
                          # layout k arrives by DMA and the update is emitted
                          # here (front stage); with transpose_k the k tiles are
                          # transposed on the PE this stage and the state update
                          # moves to the back stage so the transpose->copy->
                          # matmul chain gets a stage of slack.
                          kns0 = kns1 = None
                          if transpose_k and not probe_no_state:
                              ktp0 = ktrp.tile([C, C], F32, tag="ktr")
                              nc.tensor.transpose(ktp0[:], kcT, ident_t[:])
                              kns0 = knsp.tile([C, C], F32, tag="kns")
                              nc.vector.tensor_copy(kns0[:], ktp0[:])
                              ktp1 = ktrp.tile([C, C], F32, tag="ktr")
                              nc.tensor.transpose(ktp1[:], krcT, ident_t[:])
                              kns1 = knsp.tile([C, C], F32, tag="kns")
                              nc.scalar.copy(kns1[:], ktp1[:])
                          if not transpose_k and not probe_no_state:
                              mm(pS[p][:, 0:E1], knc, vc,
                                               start=(c == 0), stop=False,
                                               skip_group_check=True)
                              mm(pS[p][:, E1 + 1:2 * E1 + 1], krnc, vc,
                                               start=False, stop=(c == NCHUNK - 1),
                                               skip_group_check=True)
                              if c < NCHUNK - 1:
                                  s01 = ssbp.tile([D, 2 * E1 + 2], MT, tag="ssb")
                                  nc.scalar.copy(s01[:], pS[p][:])
                                  S_sbuf[p] = s01

                          # AT = K0 Q0^T + K1 Q1^T (both branches accumulate in
                          # one PSUM bank), then one causal mask (s<=t)
                          if probe_no_at:
                              atm0 = mask_t
                          else:
                              pat0 = patp.tile([C, C], F32, tag="pat")
                              if f32r:
                                  mm(pat0[:], kcT.bitcast(F32R),
                                     qcT.bitcast(F32R), start=True, stop=False)
                                  mm(pat0[:], krcT.bitcast(F32R),
                                     qrcT.bitcast(F32R), start=False, stop=True)
                              else:
                                  mm(pat0[:], kcT, qcT, start=True, stop=False)
                                  mm(pat0[:], krcT, qrcT, start=False, stop=True)
                              atm0 = atmp.tile([C, C], MT, tag="atm")
                              nc.vector.tensor_mul(atm0[:], pat0[:], mask_t[:])

                          pending[p] = dict(atm=atm0, qcT=qcT, qrcT=qrcT,
                                            vc=vc, st=st, j=j, c=c,
                                            prev_S=prev_S, kns0=kns0, kns1=kns1,
                                            c0=c0, slen=st.get("len", SLAB))

                    if ilv and cc < NCHUNK and not dma_only and not probe_pe_only:
                        ps = sorted(pending.keys())
                        # state matmuls, pair-interleaved (consecutive MMs hit
                        # different PSUM banks)
                        for br in range(2):
                            for p in ps:
                                z = pending[p]
                                if br == 0:
                                    mm(pS[p][:, 0:E1], z["knc"], z["vc"],
                                       start=(c == 0), stop=False,
                                       skip_group_check=True)
                                else:
                                    mm(pS[p][:, E1 + 1:2 * E1 + 1], z["krnc"],
                                       z["vc"], start=False,
                                       stop=(c == NCHUNK - 1),
                                       skip_group_check=True)
                        for p in ps:
                            if c < NCHUNK - 1:
                                s01 = ssbp.tile([D, 2 * E1 + 2], MT, tag="ssb",
                                                name=f"s01i_{rep}_{p}_{c}")
                                nc.scalar.copy(s01[:], pS[p][:])
                                S_sbuf[p] = s01
                        pats = {}
                        for p in ps:
                            pats[p] = patp.tile([C, C], F32, tag="pat",
                                                name=f"pati_{rep}_{p}_{c}")
                        for br in range(2):
                            for p in ps:
                                z = pending[p]
                                if br == 0:
                                    mm(pats[p][:], z["kcT"], z["qcT"],
                                       start=True, stop=False,
                                       skip_group_check=True)
                                else:
                                    mm(pats[p][:], z["krcT"], z["qrcT"],
                                       start=False, stop=True,
                                       skip_group_check=True)
                        for p in ps:
                            atm = atmp.tile([C, C], MT, tag="atm",
                                            name=f"atmi_{rep}_{p}_{c}")
                            nc.vector.tensor_mul(atm[:], pats[p][:], mask_t[:])
                            pending[p]["atm"] = atm

                    if ilv:
                        items = sorted(back.items())
                        pos = {}
                        for p, z in items:
                            pos[p] = poutp.tile([C, E1], F32, tag="po",
                                                name=f"poi_{rep}_{p}_{z['c']}")
                        for p, z in items:
                            mm(pos[p][:], z["atm"][:], z["vc"], start=True,
                               stop=(z["c"] == 0 or z["prev_S"] is None),
                               skip_group_check=True)
                        for p, z in items:
                            if z["c"] > 0 and z["prev_S"] is not None:
                                mm(pos[p][:], z["qcT"], z["prev_S"][:, 0:E1],
                                   start=False, stop=False,
                                   skip_group_check=True)
                        for p, z in items:
                            if z["c"] > 0 and z["prev_S"] is not None:
                                mm(pos[p][:], z["qrcT"],
                                   z["prev_S"][:, E1 + 1:2 * E1 + 1],
                                   start=False, stop=True,
                                   skip_group_check=True)
                        for p, z in items:
                            po = pos[p]
                            dinv = dinvp.tile([C, 1], F32, tag="dinv",
                                              name=f"dinvi_{rep}_{p}_{z['c']}")
                            nc.vector.reciprocal(dinv[:], po[:, E:E1])
                            nc.scalar.mul(z["st"]["outs"][:, z["j"], :],
                                          po[:, 0:E], dinv[:])
                            if z["j"] == z["slen"] - 1:
                                base = p * N + z["c0"] * C
                                rows = slice(base, base + z["slen"] * C)
                                nc.sync.dma_start(
                                    out[rows, :].rearrange(
                                        "(n p) e -> p n e", p=C),
                                    z["st"]["outs"][:])
                        back = {}

                    for p, z in back.items():
                        cb = z["c"]
                        # with transpose_k the state update happens here, so
                        # the pre-update state must be captured here as well
                        if transpose_k:
                            z["prev_S"] = S_sbuf.get(p)
                        if transpose_k and z["kns0"] is not None:
                            mm(pS[p][:, 0:E1], z["kns0"][:],
                                             z["vc"], start=(cb == 0),
                                             stop=False, skip_group_check=True)
                            mm(pS[p][:, E1 + 1:2 * E1 + 1],
                                             z["kns1"][:], z["vc"],
                                             start=False,
                                             stop=(cb == NCHUNK - 1),
                                             skip_group_check=True)
                            if cb < NCHUNK - 1:
                                s01 = ssbp.tile([D, 2 * E1 + 2], MT, tag="ssb")
                                nc.scalar.copy(s01[:], pS[p][:])
                                S_sbuf[p] = s01
                        # numerator (cols 0..63) + denominator (col 64)
                        po = poutp.tile([C, E1], F32, tag="po")
                        mm(po[:], z["atm"][:], z["vc"],
                                         start=True,
                                         stop=(cb == 0 or z["prev_S"] is None))
                        if cb > 0 and z["prev_S"] is not None:
                            mm(po[:], z["qcT"],
                                             z["prev_S"][:, 0:E1],
                                             start=False, stop=False,
                                             skip_group_check=True)
                            mm(po[:], z["qrcT"],
                                             z["prev_S"][:, E1 + 1:2 * E1 + 1],
                                             start=False, stop=True,
                                             skip_group_check=True)

                        if host_norm:
                            # ship numerator and denominator; host divides
                            nc.scalar.copy(z["st"]["outs"][:, z["j"], :],
                                           po[:, 0:E1])
                        else:
                            # out[t,:] = num[t,:] / den[t]
                            dinv = dinvp.tile([C, 1], F32, tag="dinv")
                            nc.vector.reciprocal(dinv[:], po[:, E:E1])
                            nc.scalar.mul(z["st"]["outs"][:, z["j"], :],
                                          po[:, 0:E], dinv[:])

                        if z["j"] == z["slen"] - 1:
                            base = p * N + z["c0"] * C
                            rows = slice(base, base + z["slen"] * C)
                            nc.sync.dma_start(
                                out[rows, :].rearrange("(n p) e -> p n e", p=C),
                                z["st"]["outs"][:])

    nc.compile()
    return nc




# Column strides inside shared PSUM banks (8-byte aligned regions)
PW = 72            # per-pair region width in the output bank (>= E1)
SW = 66            # per-(pair,branch) region width in the state bank (>= E1)


def build_kernel_m(repeat=1, loop_k=None):
    """Pair-merged variant: both (b,h) pairs handled per core share single
    PSUM banks for AT, numerator/denominator, and state, so the causal mask,
    the state evacuation, and the reciprocal each run as ONE wide
    vector/scalar op per chunk instead of one per pair. Cuts the DVE/ACT
    instruction count (and their fixed per-op drain cost) roughly in half."""
    nc = bacc.Bacc("TRN2", target_bir_lowering=False, debug=False,
                   num_devices=N_CORES)

    MT = F32  # typed-f32r rejected by walrus codegen (odd-N ISA check)

    def mm(out_ap, lhsT_ap, rhs_ap, **kw):
        if mm_f32r:
            lhsT_ap = lhsT_ap.bitcast(F32R)
            rhs_ap = rhs_ap.bitcast(F32R)
        return nc.tensor.matmul(out_ap, lhsT_ap, rhs_ap, **kw)

    qT = nc.dram_tensor("qT", [D, NROWS], MT, kind="ExternalInput").ap()
    kT = nc.dram_tensor("kT", [D, NROWS], MT, kind="ExternalInput").ap()
    qrT = nc.dram_tensor("qrT", [D, NROWS], MT, kind="ExternalInput").ap()
    krT = nc.dram_tensor("krT", [D, NROWS], MT, kind="ExternalInput").ap()
    kn = nc.dram_tensor("kn", [NROWS, D], MT, kind="ExternalInput").ap()
    krn = nc.dram_tensor("krn", [NROWS, D], MT, kind="ExternalInput").ap()
    v1 = nc.dram_tensor("v1", [NROWS, E1], MT, kind="ExternalInput").ap()
    mask2 = nc.dram_tensor("mask2", [C, 2 * C], F32, kind="ExternalInput").ap()
    out = nc.dram_tensor("out", [NROWS, E], F32, kind="ExternalOutput").ap()

    NP = PAIRS_PER_CORE  # 2

    with tile.TileContext(nc) as tc:
        with (
            tc.tile_pool(name="const", bufs=1) as constp,
            tc.tile_pool(name="slabs", bufs=6) as slabs,
            tc.tile_pool(name="atm", bufs=3) as atmp,
            tc.tile_pool(name="ssb", bufs=4) as ssbp,
            tc.tile_pool(name="dinv", bufs=8) as dinvp,
            tc.tile_pool(name="pat", bufs=3, space="PSUM") as patp,
            tc.tile_pool(name="pout", bufs=3, space="PSUM") as poutp,
            tc.tile_pool(name="pst", bufs=1, space="PSUM") as pstp,
        ):
            mask_t = constp.tile([C, 2 * C], F32, tag="mask")
            nc.sync.dma_start(mask_t[:], mask2[:])

            for rep in range(repeat):
              with (tc.For_i(0, loop_k, 1, hint_engines=(
                        mybir.EngineType.PE, mybir.EngineType.DVE,
                        mybir.EngineType.Activation, mybir.EngineType.SP))
                    if (loop_k is not None and loop_k > 1)
                    else _nullctx()):
                  # one state bank: region (p, br) at cols (2p+br)*SW
                  pSt = pstp.tile([D, 2 * NP * SW], F32, tag="pS",
                                  name=f"pSm_{rep}")

                  slab_t = [None] * NP
                  S_sbuf = [None]     # boxed: current [D, 4*SW] sbuf state

                  pending = None
                  for cc in range(NCHUNK + 1):
                    back = pending
                    pending = None
                    if cc < NCHUNK:
                      c = cc
                      sl = {}
                      for p in range(NP):
                          if c % SLAB == 0:
                              base = p * N + c * C
                              cols = slice(base, base + SLAB * C)
                              st = {}
                              st["qT"] = slabs.tile([D, slen * C], F32, tag="qT", name=f"qTs_{rep}_{p}_{c}")
                              nc.sync.dma_start(st["qT"][:], qT[:, cols])
                              st["kT"] = slabs.tile([D, slen * C], F32, tag="kT", name=f"kTs_{rep}_{p}_{c}")
                              nc.sync.dma_start(st["kT"][:], kT[:, cols])
                              st["qrT"] = slabs.tile([D, slen * C], F32, tag="qrT", name=f"qrTs_{rep}_{p}_{c}")
                              nc.sync.dma_start(st["qrT"][:], qrT[:, cols])
                              st["krT"] = slabs.tile([D, slen * C], F32, tag="krT", name=f"krTs_{rep}_{p}_{c}")
                              nc.sync.dma_start(st["krT"][:], krT[:, cols])
                              st["kn"] = slabs.tile([C, slen, D], F32, tag="kn", name=f"kns_{rep}_{p}_{c}")
                              nc.sync.dma_start(
                                  st["kn"][:],
                                  kn[cols, :].rearrange("(n p) d -> p n d", p=C))
                              st["krn"] = slabs.tile([C, slen, D], F32, tag="krn", name=f"krns_{rep}_{p}_{c}")
                              nc.sync.dma_start(
                                  st["krn"][:],
                                  krn[cols, :].rearrange("(n p) d -> p n d", p=C))
                              st["v1"] = slabs.tile([C, slen, E1], F32, tag="v1", name=f"v1s_{rep}_{p}_{c}")
                              nc.sync.dma_start(
                                  st["v1"][:],
                                  v1[cols, :].rearrange("(n p) e -> p n e", p=C))
                              st["outs"] = slabs.tile([C, SLAB, E], F32, tag="outs", name=f"outs_{rep}_{p}_{c}")
                              slab_t[p] = st

                          st = slab_t[p]
                          j = c - c0
                          sl[p] = dict(
                              st=st, j=j,
                              qcT=st["qT"][:, j * C:(j + 1) * C],
                              kcT=st["kT"][:, j * C:(j + 1) * C],
                              qrcT=st["qrT"][:, j * C:(j + 1) * C],
                              krcT=st["krT"][:, j * C:(j + 1) * C],
                              knc=st["kn"][:, j, :],
                              krnc=st["krn"][:, j, :],
                              vc=st["v1"][:, j, :],
                          )

                      prev_S = S_sbuf[0]

                      # state updates, all four into one bank
                      for p in range(NP):
                          z = sl[p]
                          nc.tensor.matmul(
                              pSt[:, (2 * p) * SW:(2 * p) * SW + E1],
                              z["knc"], z["vc"],
                              start=(c == 0 and p == 0), stop=False,
                              skip_group_check=True)
                          nc.tensor.matmul(
                              pSt[:, (2 * p + 1) * SW:(2 * p + 1) * SW + E1],
                              z["krnc"], z["vc"],
                              start=False,
                              stop=(c == NCHUNK - 1 and p == NP - 1),
                              skip_group_check=True)
                      if c < NCHUNK - 1:
                          s01 = ssbp.tile([D, 2 * NP * SW], F32, tag="ssb")
                          nc.scalar.copy(s01[:], pSt[:])
                          S_sbuf[0] = s01

                      # AT for both pairs into one bank, one mask op
                      patb = patp.tile([C, 2 * C], F32, tag="pat")
                      for p in range(NP):
                          z = sl[p]
                          reg = patb[:, p * C:(p + 1) * C]
                          nc.tensor.matmul(reg, z["kcT"], z["qcT"],
                                           start=True, stop=False,
                                           skip_group_check=True)
                          nc.tensor.matmul(reg, z["krcT"], z["qrcT"],
                                           start=False, stop=True,
                                           skip_group_check=True)
                      atm = atmp.tile([C, 2 * C], F32, tag="atm")
                      nc.vector.tensor_mul(atm[:], patb[:], mask_t[:])

                      pending = dict(atm=atm, sl=sl, c=c, prev_S=prev_S)

                    if back is not None:
                        cb = back["c"]
                        pob = poutp.tile([C, NP * PW], F32, tag="po")
                        for p in range(NP):
                            z = back["sl"][p]
                            reg = pob[:, p * PW:p * PW + E1]
                            only = (cb == 0)
                            nc.tensor.matmul(
                                reg, back["atm"][:, p * C:(p + 1) * C],
                                z["vc"], start=True, stop=only,
                                skip_group_check=True)
                            if cb > 0:
                                pv = back["prev_S"]
                                nc.tensor.matmul(
                                    reg, z["qcT"],
                                    pv[:, (2 * p) * SW:(2 * p) * SW + E1],
                                    start=False, stop=False,
                                    skip_group_check=True)
                                nc.tensor.matmul(
                                    reg, z["qrcT"],
                                    pv[:, (2 * p + 1) * SW:(2 * p + 1) * SW + E1],
                                    start=False, stop=True,
                                    skip_group_check=True)

                        # one reciprocal for both pairs' denominators
                        dinv = dinvp.tile([C, NP], F32, tag="dinv")
                        nc.vector.reciprocal(
                            dinv[:], pob[:, E:NP * PW:PW])
                        for p in range(NP):
                            z = back["sl"][p]
                            nc.scalar.mul(z["st"]["outs"][:, z["j"], :],
                                          pob[:, p * PW:p * PW + E],
                                          dinv[:, p:p + 1])
                            if z["j"] == SLAB - 1:
                                base = p * N + (cb - SLAB + 1) * C
                                rows = slice(base, base + SLAB * C)
                                nc.sync.dma_start(
                                    out[rows, :].rearrange(
                                        "(n p) e -> p n e", p=C),
                                    z["st"]["outs"][:])

    nc.compile()
    return nc



def _prepare_in_maps(q, k, q_rot, k_rot, v, transpose_k=False, merged=False,
                     dt16=True):
    b, h, n, d = q.shape
    e = v.shape[-1]
    nbh = b * h
    ht = np.float16 if dt16 else np.float32
    qf = np.ascontiguousarray(q.reshape(nbh, n, d).astype(ht))
    kf = np.ascontiguousarray(k.reshape(nbh, n, d).astype(ht))
    qrf = np.ascontiguousarray(q_rot.reshape(nbh, n, d).astype(ht))
    krf = np.ascontiguousarray(k_rot.reshape(nbh, n, d).astype(ht))
    vf = np.ascontiguousarray(v.reshape(nbh, n, e).astype(ht))
    mask = np.triu(np.ones((C, C), dtype=np.float32))

    in_maps = []
    for i in range(N_CORES):
        sel = [PAIRS_PER_CORE * i + p for p in range(PAIRS_PER_CORE)]
        qT = np.ascontiguousarray(
            np.concatenate([qf[s].T for s in sel], axis=1))
        kT = np.ascontiguousarray(
            np.concatenate([kf[s].T for s in sel], axis=1))
        qrT = np.ascontiguousarray(
            np.concatenate([qrf[s].T for s in sel], axis=1))
        krT = np.ascontiguousarray(
            np.concatenate([krf[s].T for s in sel], axis=1))
        knat = np.ascontiguousarray(np.concatenate([kf[s] for s in sel], axis=0))
        krnat = np.ascontiguousarray(np.concatenate([krf[s] for s in sel], axis=0))
        vcat = np.concatenate([vf[s] for s in sel], axis=0)
        v1 = np.ascontiguousarray(
            np.concatenate([vcat, np.ones((vcat.shape[0], 1), ht)],
                           axis=1))
        m = dict(qT=qT, kT=kT, qrT=qrT, krT=krT, v1=v1)
        if merged:
            m["mask2"] = np.ascontiguousarray(np.concatenate([mask, mask], axis=1))
        else:
            m["mask"] = mask
        if transpose_k:
            m["ident"] = np.eye(C, dtype=np.float32)
        else:
            m["kn"] = knat
            m["krn"] = krnat
        in_maps.append(m)
    return in_maps


def kernel(q, k, q_rot, k_rot, v, horizon=128, **run_kwargs):
    q = np.asarray(q)
    k = np.asarray(k)
    q_rot = np.asarray(q_rot)
    k_rot = np.asarray(k_rot)
    v = np.asarray(v)
    b, h, n, d = q.shape
    e = v.shape[-1]
    assert (b * h, n, d, e) == (N_CORES * PAIRS_PER_CORE, N, D, E), \
        "kernel is hardcoded for b*h=16, n=2048, d=128, e=64"

    if "nc" not in _cached:
        _cached["nc"] = build_kernel()
    nc = _cached["nc"]

    in_maps = _prepare_in_maps(q, k, q_rot, k_rot, v)
    res = run_bass_kernel_spmd(nc, in_maps, core_ids=list(range(N_CORES)),
                               **run_kwargs)

    outf = np.empty((b * h, n, e), dtype=np.float32)
    for i in range(N_CORES):
        o = res.results[i]["out"].reshape(PAIRS_PER_CORE, n, e)
        for p in range(PAIRS_PER_CORE):
            outf[PAIRS_PER_CORE * i + p] = o[p].astype(np.float32)
    if run_kwargs:
        kernel.last_results = res
    return outf.reshape(b, h, n, e)


if __name__ == "__main__":
    rng = np.random.default_rng(0)
    q = rng.random((2, 8, N, D), dtype=np.float32)
    k = rng.random((2, 8, N, D), dtype=np.float32)
    qr = rng.standard_normal((2, 8, N, D), dtype=np.float32)
    kr = rng.standard_normal((2, 8, N, D), dtype=np.float32)
    v = rng.random((2, 8, N, E), dtype=np.float32)
    o = kernel(q, k, qr, kr, v, 128)
    print("ok", o.shape, o.dtype, np.abs(o).mean())



# revision 7
# speedup vs baseline: 1.4873x; 1.4873x over previous
"""Trainium2 Bass kernel for chunked recurrent causal linear attention.

Problem: b=2, h=8, n=2048, d=128, e=64, chunk=128, two branches (plain +
rotary) sharing one denominator.

Math (per (b,h), per chunk c, token t in chunk, with running state
S[d,e], Z[d] per branch):
    AT[s,t]   = k_s . q_t                  (s,t in chunk; masked to s<=t)
    num[t,:]  = sum_s ATm[s,t] v_s + q_t @ S      (both branches summed)
    den[t]    = sum_s ATm[s,t]   + q_t . Z        (both branches summed)
    out[t,:]  = num[t,:] / den[t]
    S += k_chunk^T v_chunk ;  Z += sum_s k_s

Sharding: 16 (b,h) pairs over 8 cores, 2 pairs per core.

Implementation notes (v2):
  - All inputs in fp16: 2x less DMA traffic and 4x PE matmul throughput
    vs fp32 (fp32 matmuls lower to 2 half-speed passes). PSUM accumulation
    stays fp32. Measured end-to-end rel err 4.3e-4 vs the 2e-2 gate.
  - Host packs every per-chunk operand (qT/kT/qrT/krT pre-transposed,
    kn/krn natural for the state update, v plus a ones column) for both
    pairs into one [128, GW] group per CG chunks, so each input DMA is a
    single contiguous ~860KB transfer (~78% of peak vs ~30% for the old
    per-tensor 65-128KB transfers).
  - Output is written in SBUF-native layout [token-in-chunk, chunk, e]
    (contiguous 1KB-per-partition runs; fp16 rows in token-major order
    would be 128B runs, below the 512B DMA line-rate minimum) and
    inverse-permuted on host.
  - Both pairs share single PSUM banks for AT, num/den, and state, so the
    causal mask, state evacuation, and reciprocal run as ONE wide op per
    chunk instead of one per pair (halves DVE/ACT instruction count).
"""

import contextlib
import sys

_nullctx = contextlib.nullcontext

if "/opt/trn_rl_repo" not in sys.path:
    sys.path.insert(0, "/opt/trn_rl_repo")

import numpy as np

import concourse.bass as bass
import concourse.tile as tile
from concourse import bacc, mybir
from concourse.bass_utils import run_bass_kernel_spmd

F32 = mybir.dt.float32
F16 = mybir.dt.float16

N_CORES = 8
NP = 2             # (b,h) pairs per core
N = 2048           # sequence length per (b,h)
D = 128            # qk head dim
E = 64             # v head dim
E1 = E + 1         # v plus ones column
C = 128            # chunk size
NCHUNK = N // C    # 16

# input group packing: CG chunks x both pairs per DMA
CG = 2                      # chunks per group (per pair)
NG = NCHUNK // CG           # 8 groups
CW = 840                    # padded cols per (pair, chunk) section (16B align)
OFF_QT, OFF_KT, OFF_QRT, OFF_KRT = 0, 128, 256, 384
OFF_KN, OFF_KRN, OFF_V1 = 512, 640, 768
GW = NP * CG * CW           # 3360 cols = 6720B/partition per group

SW = 66            # state-bank region stride per (pair, branch) (>= E1)
PW = 72            # pout-bank region stride per pair (>= E1)
OSL = 8            # chunks per output slab
NOS = NCHUNK // OSL

_cached = {}


def build_kernel(repeat=1, loop_k=None, gbufs=4):
    nc = bacc.Bacc("TRN2", target_bir_lowering=False, debug=False,
                   num_devices=N_CORES)

    in_all = nc.dram_tensor("in_all", [NG * C, GW], F16,
                            kind="ExternalInput").ap()
    mask2 = nc.dram_tensor("mask2", [C, 2 * C], F32,
                           kind="ExternalInput").ap()
    out = nc.dram_tensor("out", [NP * NOS * C, OSL * E], F16,
                         kind="ExternalOutput").ap()

    with tile.TileContext(nc) as tc:
        with (
            tc.tile_pool(name="const", bufs=1) as constp,
            tc.tile_pool(name="grp", bufs=gbufs) as grpp,
            tc.tile_pool(name="atm", bufs=3) as atmp,
            tc.tile_pool(name="ssb", bufs=4) as ssbp,
            tc.tile_pool(name="dinv", bufs=8) as dinvp,
            tc.tile_pool(name="outs", bufs=2 * NP) as outsp,
            tc.tile_pool(name="pat", bufs=3, space="PSUM") as patp,
            tc.tile_pool(name="pout", bufs=3, space="PSUM") as poutp,
            tc.tile_pool(name="pst", bufs=1, space="PSUM") as pstp,
        ):
            mask_t = constp.tile([C, 2 * C], F32, tag="mask")
            nc.sync.dma_start(mask_t[:], mask2[:])

            for rep in range(repeat):
              with (tc.For_i(0, loop_k, 1, hint_engines=(
                        mybir.EngineType.PE, mybir.EngineType.DVE,
                        mybir.EngineType.Activation, mybir.EngineType.SP))
                    if (loop_k is not None and loop_k > 1)
                    else _nullctx()):
                # one state bank: region (p, br) at cols (2p+br)*SW
                pst = pstp.tile([D, 2 * NP * SW], F32, tag="pS",
                                name=f"pS_{rep}")

                group_tiles = {}
                S_box = [None]        # current [D, 4*SW] fp16 sbuf state
                outs_t = {}           # pair -> current output slab tile

                # Software pipeline, one chunk deep: the front stage of
                # chunk c emits the group load (every CG chunks), the state
                # update (PE), and AT+mask (PE then DVE); the back stage
                # consumes chunk c-1's masked AT for the num/den matmuls,
                # reciprocal and output scale. Every cross-engine hop gets
                # a full stage of slack.
                pending = None
                for cc in range(NCHUNK + 1):
                    back = pending
                    pending = None
                    if cc < NCHUNK:
                        c = cc
                        g, j = divmod(c, CG)
                        if j == 0:
                            gtile = grpp.tile([C, GW], F16, tag="grp",
                                              name=f"g_{rep}_{g}")
                            nc.sync.dma_start(gtile[:],
                                              in_all[g * C:(g + 1) * C, :])
                            group_tiles[g] = gtile
                        gtile = group_tiles[g]

                        def sec(p, off, w, _j=j, _g=gtile):
                            b = (p * CG + _j) * CW + off
                            return _g[:, b:b + w]

                        sl = {}
                        for p in range(NP):
                            sl[p] = dict(
                                qcT=sec(p, OFF_QT, C),
                                kcT=sec(p, OFF_KT, C),
                                qrcT=sec(p, OFF_QRT, C),
                                krcT=sec(p, OFF_KRT, C),
                                knc=sec(p, OFF_KN, D),
                                krnc=sec(p, OFF_KRN, D),
                                vc=sec(p, OFF_V1, E1),
                            )
                        if c % OSL == 0:
                            for p in range(NP):
                                outs_t[p] = outsp.tile(
                                    [C, OSL * E], F16, tag="outs",
                                    name=f"o_{rep}_{p}_{c}")

                        prev_S = S_box[0]

                        # state update: all four (pair, branch) regions in
                        # one PSUM bank, accumulated across chunks
                        for br in range(2):
                            for p in range(NP):
                                z = sl[p]
                                kin = z["knc"] if br == 0 else z["krnc"]
                                nc.tensor.matmul(
                                    pst[:, (2 * p + br) * SW:
                                        (2 * p + br) * SW + E1],
                                    kin, z["vc"],
                                    start=(c == 0 and br == 0 and p == 0),
                                    stop=(c == NCHUNK - 1 and br == 1
                                          and p == NP - 1),
                                    skip_group_check=True)
                        if c < NCHUNK - 1:
                            s01 = ssbp.tile([D, 2 * NP * SW], F16, tag="ssb",
                                            name=f"s_{rep}_{c}")
                            nc.scalar.copy(s01[:], pst[:])
                            S_box[0] = s01

                        # AT for both pairs/branches into one bank, one mask
                        patb = patp.tile([C, 2 * C], F32, tag="pat",
                                         name=f"pat_{rep}_{c}")
                        for br in range(2):
                            for p in range(NP):
                                z = sl[p]
                                kk = z["kcT"] if br == 0 else z["krcT"]
                                qq = z["qcT"] if br == 0 else z["qrcT"]
                                nc.tensor.matmul(
                                    patb[:, p * C:(p + 1) * C], kk, qq,
                                    start=(br == 0 and p == 0),
                                    stop=(br == 1 and p == NP - 1),
                                    skip_group_check=True)
                        atm = atmp.tile([C, 2 * C], F16, tag="atm",
                                        name=f"atm_{rep}_{c}")
                        nc.vector.tensor_mul(atm[:], patb[:], mask_t[:])

                        pending = dict(atm=atm, sl=sl, c=c, prev_S=prev_S,
                                       outs=dict(outs_t))

                    if back is not None:
                        cb = back["c"]
                        pob = poutp.tile([C, NP * PW], F32, tag="po",
                                         name=f"po_{rep}_{cb}")
                        first = back["prev_S"] is None
                        for p in range(NP):
                            z = back["sl"][p]
                            nc.tensor.matmul(
                                pob[:, p * PW:p * PW + E1],
                                back["atm"][:, p * C:(p + 1) * C], z["vc"],
                                start=(p == 0),
                                stop=(first and p == NP - 1),
                                skip_group_check=True)
                        if not first:
                            pv = back["prev_S"]
                            for br in range(2):
                                for p in range(NP):
                                    z = back["sl"][p]
                                    qq = z["qcT"] if br == 0 else z["qrcT"]
                                    nc.tensor.matmul(
                                        pob[:, p * PW:p * PW + E1], qq,
                                        pv[:, (2 * p + br) * SW:
                                           (2 * p + br) * SW + E1],
                                        start=False,
                                        stop=(br == 1 and p == NP - 1),
                                        skip_group_check=True)

                        # one reciprocal for both pairs' denominators
                        dinv = dinvp.tile([C, NP], F32, tag="dinv",
                                          name=f"di_{rep}_{cb}")
                        nc.vector.reciprocal(dinv[:], pob[:, E:NP * PW:PW])
                        jo = cb % OSL
                        for p in range(NP):
                            nc.scalar.mul(
                                back["outs"][p][:, jo * E:(jo + 1) * E],
                                pob[:, p * PW:p * PW + E], dinv[:, p:p + 1])
                        if jo == OSL - 1:
                            sb = cb // OSL
                            for p in range(NP):
                                r0 = (p * NOS + sb) * C
                                nc.sync.dma_start(out[r0:r0 + C, :],
                                                  back["outs"][p][:])

    nc.compile()
    return nc


def _prepare_in_maps(q, k, q_rot, k_rot, v):
    b, h, n, d = q.shape
    e = v.shape[-1]
    nbh = b * h
    ht = np.float16
    qf = np.asarray(q).reshape(nbh, n, d).astype(ht)
    kf = np.asarray(k).reshape(nbh, n, d).astype(ht)
    qrf = np.asarray(q_rot).reshape(nbh, n, d).astype(ht)
    krf = np.asarray(k_rot).reshape(nbh, n, d).astype(ht)
    vf = np.asarray(v).reshape(nbh, n, e).astype(ht)
    mask2 = np.ascontiguousarray(
        np.tile(np.triu(np.ones((C, C), np.float32)), (1, 2)))

    in_maps = []
    for i in range(N_CORES):
        sel = [NP * i + p for p in range(NP)]
        in_all = np.zeros((NG * C, GW), ht)
        for p, s in enumerate(sel):
            for cseq in range(NCHUNK):
                g, j = divmod(cseq, CG)
                base = (p * CG + j) * CW
                rows = slice(g * C, (g + 1) * C)
                blk = slice(cseq * C, (cseq + 1) * C)
                in_all[rows, base + OFF_QT:base + OFF_QT + C] = qf[s][blk].T
                in_all[rows, base + OFF_KT:base + OFF_KT + C] = kf[s][blk].T
                in_all[rows, base + OFF_QRT:base + OFF_QRT + C] = qrf[s][blk].T
                in_all[rows, base + OFF_KRT:base + OFF_KRT + C] = krf[s][blk].T
                in_all[rows, base + OFF_KN:base + OFF_KN + D] = kf[s][blk]
                in_all[rows, base + OFF_KRN:base + OFF_KRN + D] = krf[s][blk]
                in_all[rows, base + OFF_V1:base + OFF_V1 + E] = vf[s][blk]
                in_all[rows, base + OFF_V1 + E] = 1.0
        in_maps.append(dict(in_all=in_all, mask2=mask2))
    return in_maps


def kernel(q, k, q_rot, k_rot, v, horizon=128, **run_kwargs):
    q = np.asarray(q)
    k = np.asarray(k)
    q_rot = np.asarray(q_rot)
    k_rot = np.asarray(k_rot)
    v = np.asarray(v)
    b, h, n, d = q.shape
    e = v.shape[-1]
    assert (b * h, n, d, e) == (N_CORES * NP, N, D, E), \
        "kernel is hardcoded for b*h=16, n=2048, d=128, e=64"

    if "nc" not in _cached:
        _cached["nc"] = build_kernel()
    nc = _cached["nc"]

    in_maps = _prepare_in_maps(q, k, q_rot, k_rot, v)
    res = run_bass_kernel_spmd(nc, in_maps, core_ids=list(range(N_CORES)),
                               **run_kwargs)

    outf = np.empty((b * h, n, e), dtype=np.float32)
    for i in range(N_CORES):
        o = res.results[i]["out"].reshape(NP, NOS, C, OSL, E)
        for p in range(NP):
            # [NOS, C, OSL, E] -> [NOS, OSL, C, E] -> [n, e]
            outf[NP * i + p] = (o[p].transpose(0, 2, 1, 3)
                                .reshape(n, e).astype(np.float32))
    if run_kwargs:
        kernel.last_results = res
    return outf.reshape(b, h, n, e)


if __name__ == "__main__":
    rng = np.random.default_rng(0)
    q = rng.random((2, 8, N, D), dtype=np.float32)
    k = rng.random((2, 8, N, D), dtype=np.float32)
    qr = rng.standard_normal((2, 8, N, D), dtype=np.float32)
    kr = rng.standard_normal((2, 8, N, D), dtype=np.float32)
    v = rng.random((2, 8, N, E), dtype=np.float32)
    o = kernel(q, k, qr, kr, v, 128)
    print("ok", o.shape, o.dtype, np.abs(o).mean())
